# revision 45
# baseline (speedup 1.0000x reference)
"""Multi-head attention kernel for 8 Trainium2 NeuronCores.

Problem: embeddings [4, 2048, 1024], 16 heads x 64 dim, torch nn.Linear
convention (x @ W.T + b) for Q/K/V/O projections.

Sharding: batch (4) x head-halves (2) -> 8 cores. Core c handles batch
c//2, local heads (c%2)*8..(c%2)*8+8. Output projection is row-sharded;
host sums the two partial outputs per batch element and adds the
effective output bias bo' = bo + wo @ bv: softmax rows sum to 1, so
the V bias passes through attention unchanged and is folded into the
output bias on the host (16 per-core bias matmuls + one DMA deleted).

Per-core dataflow (feature dims on partitions; PE stream columns are
the wall at ~2.35 cols/ns, so every column saved counts):
  xT [1024e, 2048t] bf16 (host pre-transposed + cast)
  QT/KT [(h,d)=512, t] via PE, bias added on DVE during PSUM evac.
  V [t, (h,d)] via PE (no bias; folded into bo on the host).
  Input DMAs split per first-use chunk across the sync/scalar/gpsimd
  issue queues (issue rate ~0.6-1us/DMA is the startup constraint; a
  DMA's packets spread over all 16 engines). The first projection
  groups run with their PSUM chains interleaved across idle banks.
  Per head-quad (4 heads = 2 pairs), per q-block of 512, per k-tile:
    scores_T[k,q] row-paired matmuls (2 heads share the PE array),
    exp on ScalarE (1/8 scale folded in, no max subtraction needed),
    U[(2x64),q] col-paired matmuls; exp tiles accumulate on DVE
    (bf16 in-place adds) so sumexp costs PE only 2 M=1 matmuls per
    pair at group end instead of 16x512-col ones-matmul streams.
  normalize: ScalarE-copy of each sumexp row to SBUF (the custom-DVE
  reciprocal misreads PSUM operands at ~3% error; gpsimd
  partition_all_reduce measured 6.7us/op -- too slow) -> recip ->
  gpsimd partition-broadcast -> DVE mult (deferring pair B's
  normalize into the next group was tried and regressed ~6us: the
  stall just moves into the outproj fillers).
  yT[e_out, t] = woT.T @ attn_T accumulated over 4 pair-tiles; the
  last q-block's outproj is split: its quad0 half runs as
  quad1-qb0 fillers, so the tail streams only the quad1 half plus
  a DVE add. yT is bf16 (host sums the two row-shard halves in
  fp32).
Host: out[b] = (yT[2b] + yT[2b+1]).T + bo'.
"""

import sys

sys.path.insert(0, "/opt/trn_rl_repo")

import numpy as np
import ml_dtypes

import concourse.bass as bass
import concourse.bacc as bacc
import concourse.mybir as mybir
import concourse.bass_isa as bass_isa
import concourse.tile as tile
from concourse.bass_utils import run_bass_kernel_spmd

BF16 = mybir.dt.bfloat16
F32 = mybir.dt.float32
NPBF16 = ml_dtypes.bfloat16

B, S, E = 4, 2048, 1024
H_LOC = 8          # local heads per core
D = 64             # head dim
OL = H_LOC * D     # 512 local output dim
N_CORES = 8
QB = 512           # query block (free dim of scores_T)
NQB = S // QB      # 4
NKT = S // 128     # 16 key tiles
NET = E // 128     # 8 embed tiles


def build_program():
    from contextlib import ExitStack

    nc = bacc.Bacc("TRN2", debug=False, num_devices=N_CORES)

    xT = nc.dram_tensor("xT", [E, S], BF16, kind="ExternalInput")
    wqT = nc.dram_tensor("wqT", [E, OL], BF16, kind="ExternalInput")
    wkT = nc.dram_tensor("wkT", [E, OL], BF16, kind="ExternalInput")
    wvT = nc.dram_tensor("wvT", [E, OL], BF16, kind="ExternalInput")
    woT = nc.dram_tensor("woT", [OL, E], BF16, kind="ExternalInput")
    bqc = nc.dram_tensor("bqc", [128, 4], F32, kind="ExternalInput")
    bkc = nc.dram_tensor("bkc", [128, 4], F32, kind="ExternalInput")
    yT = nc.dram_tensor("yT", [E, S], BF16, kind="ExternalOutput")

    with tile.TileContext(nc) as tc, ExitStack() as est:
        xt_p = est.enter_context(tc.tile_pool(name="xt", bufs=NET))
        wq_p = est.enter_context(tc.tile_pool(name="wq", bufs=NET))
        wk_p = est.enter_context(tc.tile_pool(name="wk", bufs=NET))
        wv_p = est.enter_context(tc.tile_pool(name="wv", bufs=NET))
        wo_p = est.enter_context(tc.tile_pool(name="wo", bufs=4))
        bias_p = est.enter_context(tc.tile_pool(name="bias", bufs=4))
        qt_p = est.enter_context(tc.tile_pool(name="qt", bufs=4))
        kt_p = est.enter_context(tc.tile_pool(name="kt", bufs=4))
        vb_p = est.enter_context(tc.tile_pool(name="vb", bufs=NKT))
        pj_p = est.enter_context(tc.tile_pool(name="pj", bufs=1, space="PSUM"))
        sc_p = est.enter_context(tc.tile_pool(name="sc", bufs=2, space="PSUM"))
        u_p = est.enter_context(tc.tile_pool(name="u", bufs=2, space="PSUM"))
        se_p = est.enter_context(tc.tile_pool(name="se", bufs=1, space="PSUM"))
        ex_p = est.enter_context(tc.tile_pool(name="ex", bufs=20))
        ac_p = est.enter_context(tc.tile_pool(name="ac", bufs=4))
        at_p = est.enter_context(tc.tile_pool(name="at", bufs=16))
        nrm_p = est.enter_context(tc.tile_pool(name="nrm", bufs=2))
        ys_p = est.enter_context(tc.tile_pool(name="ys", bufs=2))
        ya_p = est.enter_context(tc.tile_pool(name="ya", bufs=NET))
        usb_p = est.enter_context(tc.tile_pool(name="usb", bufs=4))

        # ---- load inputs ----
        # Issue-order + queue split so the first projection groups can
        # start ~8us in (one queue serializes issue at ~0.6-1us/DMA;
        # a DMA's packets spread across all 16 engines, so many small
        # DMAs in flight saturate HBM):
        #   sync:   biases, x j0-chunks, x rest, wq/wk col-rest
        #   scalar: wq/wk first col-chunk (before any exp work)
        #   gpsimd: wv (full), wo; output tiles later
        xts = [xt_p.tile([128, S], BF16, tag="xt", name="xt")
               for _ in range(NET)]
        wts = {
            name: [pool.tile([128, OL], BF16, tag="w" + name,
                             name="w" + name) for _ in range(NET)]
            for name, pool in (("q", wq_p), ("k", wk_p), ("v", wv_p))
        }
        wos = [wo_p.tile([128, E], BF16, tag="wo", name="wo")
               for _ in range(4)]
        bqs = bias_p.tile([128, 4], F32, tag="bqc")
        bks = bias_p.tile([128, 4], F32, tag="bkc")
        onecol = bias_p.tile([128, 1], BF16, tag="onecol")
        nc.vector.memset(onecol[:], 1.0)
        nc.sync.dma_start(bqs[:], bqc[:])
        nc.sync.dma_start(bks[:], bkc[:])
        # Preload the exp activation table while input DMAs run.
        warm = bias_p.tile([1, 16], F32, tag="warm")
        nc.vector.memset(warm[:], 0.0)
        warm2 = bias_p.tile([1, 16], F32, tag="warm2")
        nc.scalar.activation(warm2[:], warm[:],
                             mybir.ActivationFunctionType.Exp)
        # queue-priority order matches first use: prologue needs
        # x_j0+wq_i0+wk_i0 (~13us), wv by ~18us, x_j1..3 by steps 0-12,
        # wq/wk_i1 by step ~13, i2/i3 by ~step 36+, wo much later
        for e in range(NET):
            r = slice(e * 128, (e + 1) * 128)
            nc.scalar.dma_start(wts["q"][e][:, 0:128], wqT[r, 0:128])
        for e in range(NET):
            r = slice(e * 128, (e + 1) * 128)
            nc.scalar.dma_start(wts["k"][e][:, 0:128], wkT[r, 0:128])
        for e in range(NET):
            r = slice(e * 128, (e + 1) * 128)
            nc.gpsimd.dma_start(wts["v"][e][:], wvT[r, :])
        for j in range(4):
            for e in range(NET):
                r = slice(e * 128, (e + 1) * 128)
                nc.sync.dma_start(xts[e][:, j * QB:(j + 1) * QB],
                                  xT[r, j * QB:(j + 1) * QB])
        for e in range(NET):
            r = slice(e * 128, (e + 1) * 128)
            nc.gpsimd.dma_start(wts["q"][e][:, 128:256], wqT[r, 128:256])
        for e in range(NET):
            r = slice(e * 128, (e + 1) * 128)
            nc.gpsimd.dma_start(wts["k"][e][:, 128:256], wkT[r, 128:256])
        for e in range(NET):
            r = slice(e * 128, (e + 1) * 128)
            nc.sync.dma_start(wts["q"][e][:, 256:OL], wqT[r, 256:OL])
            nc.sync.dma_start(wts["k"][e][:, 256:OL], wkT[r, 256:OL])
        for p in range(4):
            nc.sync.dma_start(wos[p][:], woT[p * 128:(p + 1) * 128, :])

        qts = [qt_p.tile([128, S], BF16, tag="qt", name="qt")
               for _ in range(4)]
        kts = [kt_p.tile([128, S], BF16, tag="kt", name="kt")
               for _ in range(4)]
        vbs = [vb_p.tile([128, OL], BF16, tag="vb", name="vb")
               for _ in range(NKT)]
        atts = [[at_p.tile([128, QB], BF16, tag="at", name="at")
                 for _ in range(4)] for _ in range(NQB)]

        # ---- projection / outproj group emitters (PE fillers) ----
        def qk_group(i, j, which):
            """Q or K projection for o-tile i, t-block j (one PSUM group)."""
            w = wts[which]
            bias_t = bqs if which == "q" else bks
            dest = qts[i] if which == "q" else kts[i]
            acc = pj_p.tile([128, QB], F32, tag="pj", name="pj")
            for e in range(NET):
                nc.tensor.matmul(
                    acc[:],
                    w[e][:, i * 128:(i + 1) * 128],
                    xts[e][:, j * QB:(j + 1) * QB],
                    start=(e == 0), stop=(e == NET - 1),
                )
            nc.vector.tensor_scalar_add(
                dest[:, j * QB:(j + 1) * QB], acc[:], bias_t[:, i:i + 1])

        def v_group(ti):
            acc = pj_p.tile([128, OL], F32, tag="pj", name="pjv")
            for e in range(NET):
                nc.tensor.matmul(
                    acc[:],
                    xts[e][:, ti * 128:(ti + 1) * 128],
                    wts["v"][e][:],
                    start=(e == 0), stop=(e == NET - 1),
                )
            nc.vector.tensor_copy(vbs[ti][:], acc[:])

        def prologue_interleaved():
            """First groups with their PSUM chains interleaved across
            banks (sc/u pools are idle before attention starts), so
            matmuls from independent chains pipeline while the input
            DMAs are still streaming."""
            accq = pj_p.tile([128, QB], F32, tag="pj", name="pj")
            acck = sc_p.tile([128, 2 * QB], F32, tag="sc", name="sc")
            for e in range(NET):
                nc.tensor.matmul(
                    accq[:], wts["q"][e][:, 0:128], xts[e][:, 0:QB],
                    start=(e == 0), stop=(e == NET - 1))
                nc.tensor.matmul(
                    acck[:, 0:QB], wts["k"][e][:, 0:128],
                    xts[e][:, 0:QB],
                    start=(e == 0), stop=(e == NET - 1))
            nc.vector.tensor_scalar_add(
                qts[0][:, 0:QB], accq[:], bqs[:, 0:1])
            nc.vector.tensor_scalar_add(
                kts[0][:, 0:QB], acck[:, 0:QB], bks[:, 0:1])
            accv = [sc_p.tile([128, 2 * QB], F32, tag="sc", name="sc"),
                    u_p.tile([128, QB], F32, tag="u", name="uA"),
                    u_p.tile([128, QB], F32, tag="u", name="uB")]
            for e in range(NET):
                for t in range(3):
                    nc.tensor.matmul(
                        accv[t][:, 0:OL] if t == 0 else accv[t][:],
                        xts[e][:, t * 128:(t + 1) * 128],
                        wts["v"][e][:],
                        start=(e == 0), stop=(e == NET - 1),
                    )
            for t in range(3):
                nc.vector.tensor_copy(
                    vbs[t][:], accv[t][:, 0:OL] if t == 0 else accv[t][:])

        def out_dma(eo, qb, ysb):
            q = nc.sync if eo % 2 == 0 else nc.gpsimd
            q.dma_start(
                yT[eo * 128:(eo + 1) * 128, qb * QB:(qb + 1) * QB], ysb[:])

        def outproj_group(qb, eo, pool=None, tag="pj"):
            y = (pool or pj_p).tile([128, QB], F32, tag=tag, name="y")
            for p2 in range(4):
                nc.tensor.matmul(
                    y[:],
                    wos[p2][:, eo * 128:(eo + 1) * 128],
                    atts[qb][p2][:],
                    start=(p2 == 0), stop=(p2 == 3),
                )
            ysb = ys_p.tile([128, QB], BF16, tag="ys", name="ys")
            nc.vector.tensor_copy(ysb[:], y[:])
            out_dma(eo, qb, ysb)

        # last q-block's outproj is split: the quad0 half (pair-tiles
        # 0,1 are done by ~step 130) runs as fillers during quad1-qb0;
        # the tail only streams the quad1 half and adds the saved half
        ya_sbs = [None] * NET

        def outproj3_a(eo):
            y = pj_p.tile([128, QB], F32, tag="pj", name="ya")
            for p2 in range(2):
                nc.tensor.matmul(
                    y[:],
                    wos[p2][:, eo * 128:(eo + 1) * 128],
                    atts[3][p2][:],
                    start=(p2 == 0), stop=(p2 == 1),
                )
            ya = ya_p.tile([128, QB], BF16, tag="ya", name="ya_sb")
            nc.vector.tensor_copy(ya[:], y[:])
            ya_sbs[eo] = ya

        def outproj3_b1(eo):
            # pair-tile 2 is normalized ~16 steps before the end (pair
            # A of the last group): fold it into the saved half early
            # so the tail streams only pair-tile 3
            y = pj_p.tile([128, QB], F32, tag="pj", name="yb1")
            nc.tensor.matmul(
                y[:], wos[2][:, eo * 128:(eo + 1) * 128], atts[3][2][:],
                start=True, stop=True)
            nc.vector.tensor_add(ya_sbs[eo][:], ya_sbs[eo][:], y[:])

        def outproj3_b2(eo):
            y = sc_p.tile([128, QB], F32, tag="sc", name="yb")
            nc.tensor.matmul(
                y[:], wos[3][:, eo * 128:(eo + 1) * 128], atts[3][3][:],
                start=True, stop=True)
            ysb = ys_p.tile([128, QB], BF16, tag="ys", name="ys")
            nc.vector.tensor_add(ysb[:], ya_sbs[eo][:], y[:])
            out_dma(eo, 3, ysb)

        # ---- filler schedule ----
        def qg(i, j):
            return lambda: qk_group(i, j, "q")

        def kg(i, j):
            return lambda: qk_group(i, j, "k")

        def vg(t):
            return lambda: v_group(t)

        def og(qb, e):
            return lambda: outproj_group(qb, e)

        # Flat software-pipelined step list: one step per (quad, qb,
        # pair-in-quad, kt). At step i the ScalarE exp for step i is
        # emitted first, then the scores matmuls for step i+1, then PE
        # filler groups, then the U / sumexp matmuls for step i (which
        # wait on exp i) -- so ScalarE always has its next input queued.
        step_list = []
        for quad in range(2):
            for qb in range(NQB):
                for pi in range(2):
                    for kt in range(NKT):
                        step_list.append((quad, qb, 2 * quad + pi, pi, kt))
        nsteps = len(step_list)

        def sidx(quad, qb, pi, kt):
            return ((quad * NQB + qb) * 2 + pi) * NKT + kt

        sched = {}

        def put(step, th):
            sched.setdefault(step, []).append(th)

        for t in range(3, NKT):          # V(t) needed at step t
            put(t - 3, vg(t))
        for j in range(1, 4):            # K(0,j) read from step 4j-1
            put(4 * j - 4, kg(0, j))
        put(12, qg(1, 0))                # pair 1 starts at step 16
        put(13, kg(1, 0))
        for j in range(1, 4):            # K(1,j) read from step 16+4j-1
            put(16 + 4 * j - 4, kg(1, j))
        put(sidx(0, 0, 1, 12), qg(0, 1))   # Q(0,1) by step 32
        put(sidx(0, 1, 0, 12), qg(1, 1))   # Q(1,1) by step 48
        put(sidx(0, 1, 1, 4), qg(0, 2))    # Q(0,2) by step 64
        put(sidx(0, 1, 1, 12), qg(1, 2))
        put(sidx(0, 2, 0, 4), qg(0, 3))
        put(sidx(0, 2, 0, 12), qg(1, 3))
        # quad1 K/Q: needed from step 128 on. kg(2,*) sit exactly on
        # the quad0 group-end steps: those steps otherwise have no
        # filler, so the PE queue reaches the sumexp matmuls ~1.1us
        # before the DVE exp-accumulate chain finishes (16 stalls)
        put(sidx(0, 0, 1, 15), kg(2, 0))
        put(sidx(0, 1, 1, 15), kg(2, 1))
        put(sidx(0, 2, 1, 15), kg(2, 2))
        put(sidx(0, 3, 1, 15), kg(2, 3))
        put(sidx(0, 3, 0, 0), kg(3, 0))
        put(sidx(0, 3, 0, 4), kg(3, 1))
        put(sidx(0, 3, 0, 8), kg(3, 2))
        put(sidx(0, 3, 0, 12), kg(3, 3))
        put(sidx(0, 3, 1, 0), qg(2, 0))
        put(sidx(0, 3, 1, 8), qg(3, 0))
        put(sidx(1, 0, 0, 0), qg(2, 1))
        put(sidx(1, 0, 0, 8), qg(3, 1))
        put(sidx(1, 0, 1, 0), qg(2, 2))
        put(sidx(1, 0, 1, 8), qg(3, 2))
        put(sidx(1, 1, 0, 0), qg(2, 3))
        put(sidx(1, 1, 0, 8), qg(3, 3))
        for e in range(NET):             # outproj(qb) in (quad1, qb+1)
            # e=7 lands on the group-end step (kt15) as stall cover
            kt_e = 15 if e == 7 else 2 + 4 * (e // 2)
            put(sidx(1, 1, e % 2, kt_e), og(0, e))
            put(sidx(1, 2, e % 2, kt_e), og(1, e))
            put(sidx(1, 3, e % 2, 2 + 4 * (e // 2)), og(2, e))
        og3a_slots = [(0, 6), (0, 10), (0, 12), (0, 14),
                      (1, 2), (1, 6), (1, 10), (1, 15)]
        for e, (p, k) in enumerate(og3a_slots):
            put(sidx(1, 0, p, k),
                (lambda eo: (lambda: outproj3_a(eo)))(e))
        for e, k in enumerate([3, 4, 5, 7, 9, 11, 12, 13]):
            put(sidx(1, 3, 1, k),
                (lambda eo: (lambda: outproj3_b1(eo)))(e))

        prologue_interleaved()

        # ---- attention ----
        def emit_scores(quad, qb, pair, kt):
            sc = sc_p.tile([128, 2 * QB], F32, tag="sc", name="sc")
            nc.tensor.matmul(
                sc[:, 0:QB],
                kts[pair][0:64, kt * 128:(kt + 1) * 128],
                qts[pair][0:64, qb * QB:(qb + 1) * QB],
                start=True, stop=True, tile_position=(0, 0),
            )
            nc.tensor.matmul(
                sc[:, QB:2 * QB],
                kts[pair][64:128, kt * 128:(kt + 1) * 128],
                qts[pair][64:128, qb * QB:(qb + 1) * QB],
                start=True, stop=True, tile_position=(64, 0),
            )
            return sc

        def norm_pair(quad, qb, pr, u2, acc):
            """Reduce one pair's exp-sum (2 M=1 matmuls from the DVE-
            accumulated tile) and normalize its U into atts.
            (gpsimd partition_all_reduce measured 6.7us/op on HW --
            4x the model -- so the PE ones-matmul stays.)"""
            seb = se_p.tile([64, QB], F32, tag="se", name="seb")
            for sub in range(2):
                nc.tensor.matmul(
                    seb[sub * 32:sub * 32 + 1, :],
                    onecol[:],
                    acc[:, sub * QB:(sub + 1) * QB],
                    start=True, stop=True,
                    tile_position=(0, sub * 32),
                    skip_group_check=True,
                )
            for sub in range(2):
                # stage Z to SBUF on ScalarE (Copy shares the exp
                # activation table, so no table reload); the custom-DVE
                # reciprocal misreads PSUM operands (~3% error) so it
                # must read SBUF
                rcs = nrm_p.tile([1, QB], F32, tag="rcs", name="rcs")
                nc.scalar.copy(rcs[:], seb[sub * 32:sub * 32 + 1, :])
                rcr = nrm_p.tile([1, QB], F32, tag="rcr", name="rcr")
                nc.vector.reciprocal_approx_fast(rcr[:], rcs[:])
                bcf = nrm_p.tile([128, QB], F32, tag="bcf", name="bcf")
                nc.gpsimd.partition_broadcast(bcf[:], rcr[:])
                nc.vector.tensor_mul(
                    atts[qb][pr][sub * 64:(sub + 1) * 64, :],
                    u2[sub * 64:(sub + 1) * 64, :],
                    bcf[sub * 64:(sub + 1) * 64, :])

        q0, q1, p1, _, k1 = step_list[0]
        pend_sc = emit_scores(q0, q1, p1, k1)
        cur = {}      # per-(quad,qb) state: uA, uB, acc0, acc1
        for i, (quad, qb, pair, pi, kt) in enumerate(step_list):
            if (pi, kt) == (0, 0):
                cur["uA"] = u_p.tile([128, QB], F32, tag="u", name="uA")
                cur["uB"] = u_p.tile([128, QB], F32, tag="u", name="uB")
            # exp for this step
            et = ex_p.tile([128, 2 * QB], BF16, tag="ex", name="ex")
            nc.scalar.activation(
                et[:], pend_sc[:],
                mybir.ActivationFunctionType.Exp, scale=0.125)
            # scores for next step
            if i + 1 < nsteps:
                nq, nqb, npair, _, nkt = step_list[i + 1]
                pend_sc = emit_scores(nq, nqb, npair, nkt)
            # fillers
            for th in sched.get(i, []):
                th()
            # U matmuls for this step
            u = cur["uA"] if pi == 0 else cur["uB"]
            for sub in range(2):
                hcol = (pair * 2 + sub) * D
                nc.tensor.matmul(
                    u[sub * 64:(sub + 1) * 64, :],
                    vbs[kt][:, hcol:hcol + D],
                    et[:, sub * QB:(sub + 1) * QB],
                    start=(kt == 0), stop=(kt == NKT - 1),
                    tile_position=(0, sub * 64),
                    skip_group_check=True,
                )
            # sumexp: accumulate exp tiles on DVE (frees the PE from
            # 16x M=1 ones-matmul streams per pair; one small matmul
            # per pair at group end reduces the accumulated tile)
            acck = "acc1" if pi else "acc0"
            if kt == 0:
                acc = ac_p.tile([128, 2 * QB], BF16, tag="ac",
                                name=acck)
                cur[acck] = acc
                nc.vector.tensor_copy(acc[:], et[:])
            else:
                acc = cur[acck]
                nc.vector.tensor_add(acc[:], acc[:], et[:])
            if pi == 1 and kt == 0:
                # pair A's U is complete: evacuate it so its PSUM bank
                # frees long before the next q-block needs it
                ua_sb = usb_p.tile([128, QB], BF16, tag="usb", name="ua_sb")
                nc.vector.tensor_copy(ua_sb[:], cur["uA"][:])
                cur["uA_sb"] = ua_sb
                # pair A's exp sum (acc0) is also complete: reduce +
                # normalize pair A now, spreading the group-end chain
                # across the pi=1 half instead of serializing at kt=15
                norm_pair(quad, qb, 2 * quad, ua_sb, cur["acc0"])
            if pi == 1 and kt == NKT - 1:
                norm_pair(quad, qb, 2 * quad + 1, cur["uB"],
                          cur["acc1"])
        # tail: only pair-tile 3 of the last q-block's outproj remains
        # (scores pool is free by now -- its banks pipeline)
        for eo in range(NET):
            outproj3_b2(eo)

    nc.compile()
    return nc


_CACHED = {}


def _get_program():
    if "nc" not in _CACHED:
        _CACHED["nc"] = build_program()
    return _CACHED["nc"]


def make_inputs(embeddings, wq, bq, wk, bk, wv, bv, wo, bo):
    """Host-side sharding: per-core input maps."""
    in_maps = []
    for c in range(N_CORES):
        b, half = c // 2, c % 2
        sl = slice(half * OL, (half + 1) * OL)
        in_maps.append({
            "xT": np.ascontiguousarray(embeddings[b].T).astype(NPBF16),
            "wqT": np.ascontiguousarray(wq[sl, :].T).astype(NPBF16),
            "wkT": np.ascontiguousarray(wk[sl, :].T).astype(NPBF16),
            "wvT": np.ascontiguousarray(wv[sl, :].T).astype(NPBF16),
            "woT": np.ascontiguousarray(wo[:, sl].T).astype(NPBF16),
            "bqc": np.ascontiguousarray(
                bq[sl].reshape(4, 128).T).astype(np.float32),
            "bkc": np.ascontiguousarray(
                bk[sl].reshape(4, 128).T).astype(np.float32),
        })
    return in_maps


def unshard(results, bo_eff):
    out = np.empty((B, S, E), np.float32)
    for b in range(B):
        yt = (results[2 * b]["yT"].astype(np.float32)
              + results[2 * b + 1]["yT"].astype(np.float32))
        out[b] = yt.T + bo_eff[None, :]
    return out


def kernel(embeddings, wq, bq, wk, bk, wv, bv, wo, bo, _trace=False):
    embeddings = np.asarray(embeddings, np.float32)
    wv = np.asarray(wv, np.float32)
    bv = np.asarray(bv, np.float32)
    wo = np.asarray(wo, np.float32)
    bo = np.asarray(bo, np.float32)
    # V bias passes through softmax (rows sum to 1): fold into bo.
    bo_eff = bo + wo @ bv
    nc = _get_program()
    in_maps = make_inputs(
        embeddings, np.asarray(wq, np.float32), np.asarray(bq, np.float32),
        np.asarray(wk, np.float32), np.asarray(bk, np.float32),
        wv, bv, wo, bo)
    res = run_bass_kernel_spmd(
        nc, in_maps, core_ids=list(range(N_CORES)), trace=_trace)
    out = unshard(res.results, bo_eff)
    if _trace:
        kernel.last_result = res
    return out



# revision 50
# speedup vs baseline: 1.0073x; 1.0073x over previous
"""Multi-head attention kernel for 8 Trainium2 NeuronCores.

Problem: embeddings [4, 2048, 1024], 16 heads x 64 dim, torch nn.Linear
convention (x @ W.T + b) for Q/K/V/O projections.

Sharding: batch (4) x head-halves (2) -> 8 cores. Core c handles batch
c//2, local heads (c%2)*8..(c%2)*8+8. Output projection is row-sharded;
host sums the two partial outputs per batch element and adds the
effective output bias bo' = bo + wo @ bv: softmax rows sum to 1, so
the V bias passes through attention unchanged and is folded into the
output bias on the host (16 per-core bias matmuls + one DMA deleted).

Per-core dataflow (feature dims on partitions; PE stream columns are
the wall at ~2.35 cols/ns, so every column saved counts):
  xT [1024e, 2048t] bf16 (host pre-transposed + cast)
  QT/KT [(h,d)=512, t] via PE, bias added on DVE during PSUM evac.
  V [t, (h,d)] via PE (no bias; folded into bo on the host).
  Input DMAs split per first-use chunk across the sync/scalar/gpsimd
  issue queues (issue rate ~0.6-1us/DMA is the startup constraint; a
  DMA's packets spread over all 16 engines). The first projection
  groups run with their PSUM chains interleaved across idle banks.
  Per head-quad (4 heads = 2 pairs), per q-block of 512, per k-tile:
    scores_T[k,q] row-paired matmuls (2 heads share the PE array),
    exp on ScalarE (1/8 scale folded in, no max subtraction needed),
    U[(2x64),q] col-paired matmuls; exp tiles accumulate on DVE
    (bf16 in-place adds) so sumexp costs PE only 2 M=1 matmuls per
    pair at group end instead of 16x512-col ones-matmul streams.
  normalize: ScalarE-copy of each sumexp row to SBUF (the custom-DVE
  reciprocal misreads PSUM operands at ~3% error; gpsimd
  partition_all_reduce measured 6.7us/op -- too slow) -> recip ->
  gpsimd partition-broadcast -> DVE mult (deferring pair B's
  normalize into the next group was tried and regressed ~6us: the
  stall just moves into the outproj fillers).
  yT[e_out, t] = woT.T @ attn_T accumulated over 4 pair-tiles; the
  last q-block's outproj is split: its quad0 half runs as
  quad1-qb0 fillers, so the tail streams only the quad1 half plus
  a DVE add. yT is bf16 (host sums the two row-shard halves in
  fp32).
Host: out[b] = (yT[2b] + yT[2b+1]).T + bo'.
"""

import sys

sys.path.insert(0, "/opt/trn_rl_repo")

import numpy as np
import ml_dtypes

import concourse.bass as bass
import concourse.bacc as bacc
import concourse.mybir as mybir
import concourse.bass_isa as bass_isa
import concourse.tile as tile
from concourse.bass_utils import run_bass_kernel_spmd

BF16 = mybir.dt.bfloat16
F32 = mybir.dt.float32
NPBF16 = ml_dtypes.bfloat16

B, S, E = 4, 2048, 1024
H_LOC = 8          # local heads per core
D = 64             # head dim
OL = H_LOC * D     # 512 local output dim
N_CORES = 8
QB = 512           # query block (free dim of scores_T)
NQB = S // QB      # 4
NKT = S // 128     # 16 key tiles
NET = E // 128     # 8 embed tiles


def build_program():
    from contextlib import ExitStack

    nc = bacc.Bacc("TRN2", debug=False, num_devices=N_CORES)

    xT = nc.dram_tensor("xT", [E, S], BF16, kind="ExternalInput")
    wqT = nc.dram_tensor("wqT", [E, OL], BF16, kind="ExternalInput")
    wkT = nc.dram_tensor("wkT", [E, OL], BF16, kind="ExternalInput")
    wvT = nc.dram_tensor("wvT", [E, OL], BF16, kind="ExternalInput")
    woT = nc.dram_tensor("woT", [OL, E], BF16, kind="ExternalInput")
    bqc = nc.dram_tensor("bqc", [128, 4], F32, kind="ExternalInput")
    bkc = nc.dram_tensor("bkc", [128, 4], F32, kind="ExternalInput")
    yT = nc.dram_tensor("yT", [E, S], BF16, kind="ExternalOutput")

    with tile.TileContext(nc) as tc, ExitStack() as est:
        xt_p = est.enter_context(tc.tile_pool(name="xt", bufs=NET))
        wq_p = est.enter_context(tc.tile_pool(name="wq", bufs=NET))
        wk_p = est.enter_context(tc.tile_pool(name="wk", bufs=NET))
        wv_p = est.enter_context(tc.tile_pool(name="wv", bufs=NET))
        wo_p = est.enter_context(tc.tile_pool(name="wo", bufs=4))
        bias_p = est.enter_context(tc.tile_pool(name="bias", bufs=4))
        qt_p = est.enter_context(tc.tile_pool(name="qt", bufs=4))
        kt_p = est.enter_context(tc.tile_pool(name="kt", bufs=4))
        vb_p = est.enter_context(tc.tile_pool(name="vb", bufs=NKT))
        pj_p = est.enter_context(tc.tile_pool(name="pj", bufs=1, space="PSUM"))
        sc_p = est.enter_context(tc.tile_pool(name="sc", bufs=2, space="PSUM"))
        u_p = est.enter_context(tc.tile_pool(name="u", bufs=2, space="PSUM"))
        se_p = est.enter_context(tc.tile_pool(name="se", bufs=1, space="PSUM"))
        ex_p = est.enter_context(tc.tile_pool(name="ex", bufs=20))
        ac_p = est.enter_context(tc.tile_pool(name="ac", bufs=4))
        at_p = est.enter_context(tc.tile_pool(name="at", bufs=16))
        nrm_p = est.enter_context(tc.tile_pool(name="nrm", bufs=2))
        ys_p = est.enter_context(tc.tile_pool(name="ys", bufs=2))
        ya_p = est.enter_context(tc.tile_pool(name="ya", bufs=NET))
        usb_p = est.enter_context(tc.tile_pool(name="usb", bufs=4))

        # ---- load inputs ----
        # Issue-order + queue split so the first projection groups can
        # start ~8us in (one queue serializes issue at ~0.6-1us/DMA;
        # a DMA's packets spread across all 16 engines, so many small
        # DMAs in flight saturate HBM):
        #   sync:   biases, x j0-chunks, x rest, wq/wk col-rest
        #   scalar: wq/wk first col-chunk (before any exp work)
        #   gpsimd: wv (full), wo; output tiles later
        xts = [xt_p.tile([128, S], BF16, tag="xt", name="xt")
               for _ in range(NET)]
        wts = {
            name: [pool.tile([128, OL], BF16, tag="w" + name,
                             name="w" + name) for _ in range(NET)]
            for name, pool in (("q", wq_p), ("k", wk_p), ("v", wv_p))
        }
        wos = [wo_p.tile([128, E], BF16, tag="wo", name="wo")
               for _ in range(4)]
        bqs = bias_p.tile([128, 4], F32, tag="bqc")
        bks = bias_p.tile([128, 4], F32, tag="bkc")
        onecol = bias_p.tile([128, 1], BF16, tag="onecol")
        nc.vector.memset(onecol[:], 1.0)
        nc.sync.dma_start(bqs[:], bqc[:])
        nc.sync.dma_start(bks[:], bkc[:])
        # PE clock warmup: the engine idles ~11us waiting for input
        # DMAs and then runs the whole projection phase at the low
        # p-state (~600ns/512-col matmul vs 215ns hot). Burn dummy
        # matmuls on memset scratch during the DMA window so the real
        # prologue starts on a hot clock.
        wsrc = bias_p.tile([128, QB], BF16, tag="wsrc")
        nc.vector.memset(wsrc[:], 0.0)
        wdst = se_p.tile([128, QB], F32, tag="se", name="warmmm")
        for _ in range(24):
            nc.tensor.matmul(wdst[0:1, :], onecol[:], wsrc[:],
                             start=True, stop=True)
        # Preload the exp activation table while input DMAs run.
        warm = bias_p.tile([1, 16], F32, tag="warm")
        nc.vector.memset(warm[:], 0.0)
        warm2 = bias_p.tile([1, 16], F32, tag="warm2")
        nc.scalar.activation(warm2[:], warm[:],
                             mybir.ActivationFunctionType.Exp)
        # queue-priority order matches first use: prologue needs
        # x_j0+wq_i0+wk_i0 (~13us), wv by ~18us, x_j1..3 by steps 0-12,
        # wq/wk_i1 by step ~13, i2/i3 by ~step 36+, wo much later
        for e in range(NET):
            r = slice(e * 128, (e + 1) * 128)
            nc.scalar.dma_start(wts["q"][e][:, 0:128], wqT[r, 0:128])
        for e in range(NET):
            r = slice(e * 128, (e + 1) * 128)
            nc.scalar.dma_start(wts["k"][e][:, 0:128], wkT[r, 0:128])
        for e in range(NET):
            r = slice(e * 128, (e + 1) * 128)
            nc.gpsimd.dma_start(wts["v"][e][:], wvT[r, :])
        for j in range(4):
            for e in range(NET):
                r = slice(e * 128, (e + 1) * 128)
                nc.sync.dma_start(xts[e][:, j * QB:(j + 1) * QB],
                                  xT[r, j * QB:(j + 1) * QB])
        for e in range(NET):
            r = slice(e * 128, (e + 1) * 128)
            nc.gpsimd.dma_start(wts["q"][e][:, 128:256], wqT[r, 128:256])
        for e in range(NET):
            r = slice(e * 128, (e + 1) * 128)
            nc.gpsimd.dma_start(wts["k"][e][:, 128:256], wkT[r, 128:256])
        for e in range(NET):
            r = slice(e * 128, (e + 1) * 128)
            nc.sync.dma_start(wts["q"][e][:, 256:OL], wqT[r, 256:OL])
            nc.sync.dma_start(wts["k"][e][:, 256:OL], wkT[r, 256:OL])
        for p in range(4):
            nc.sync.dma_start(wos[p][:], woT[p * 128:(p + 1) * 128, :])

        qts = [qt_p.tile([128, S], BF16, tag="qt", name="qt")
               for _ in range(4)]
        kts = [kt_p.tile([128, S], BF16, tag="kt", name="kt")
               for _ in range(4)]
        vbs = [vb_p.tile([128, OL], BF16, tag="vb", name="vb")
               for _ in range(NKT)]
        atts = [[at_p.tile([128, QB], BF16, tag="at", name="at")
                 for _ in range(4)] for _ in range(NQB)]

        # ---- projection / outproj group emitters (PE fillers) ----
        def qk_group(i, j, which):
            """Q or K projection for o-tile i, t-block j (one PSUM group)."""
            w = wts[which]
            bias_t = bqs if which == "q" else bks
            dest = qts[i] if which == "q" else kts[i]
            acc = pj_p.tile([128, QB], F32, tag="pj", name="pj")
            for e in range(NET):
                nc.tensor.matmul(
                    acc[:],
                    w[e][:, i * 128:(i + 1) * 128],
                    xts[e][:, j * QB:(j + 1) * QB],
                    start=(e == 0), stop=(e == NET - 1),
                )
            nc.vector.tensor_scalar_add(
                dest[:, j * QB:(j + 1) * QB], acc[:], bias_t[:, i:i + 1])

        def v_group(ti):
            acc = pj_p.tile([128, OL], F32, tag="pj", name="pjv")
            for e in range(NET):
                nc.tensor.matmul(
                    acc[:],
                    xts[e][:, ti * 128:(ti + 1) * 128],
                    wts["v"][e][:],
                    start=(e == 0), stop=(e == NET - 1),
                )
            nc.vector.tensor_copy(vbs[ti][:], acc[:])

        def prologue_interleaved():
            """First groups with their PSUM chains interleaved across
            banks (sc/u pools are idle before attention starts), so
            matmuls from independent chains pipeline while the input
            DMAs are still streaming."""
            accq = pj_p.tile([128, QB], F32, tag="pj", name="pj")
            acck = sc_p.tile([128, 2 * QB], F32, tag="sc", name="sc")
            for e in range(NET):
                nc.tensor.matmul(
                    accq[:], wts["q"][e][:, 0:128], xts[e][:, 0:QB],
                    start=(e == 0), stop=(e == NET - 1))
                nc.tensor.matmul(
                    acck[:, 0:QB], wts["k"][e][:, 0:128],
                    xts[e][:, 0:QB],
                    start=(e == 0), stop=(e == NET - 1))
            nc.vector.tensor_scalar_add(
                qts[0][:, 0:QB], accq[:], bqs[:, 0:1])
            nc.vector.tensor_scalar_add(
                kts[0][:, 0:QB], acck[:, 0:QB], bks[:, 0:1])
            accv = [sc_p.tile([128, 2 * QB], F32, tag="sc", name="sc"),
                    u_p.tile([128, QB], F32, tag="u", name="uA"),
                    u_p.tile([128, QB], F32, tag="u", name="uB")]
            for e in range(NET):
                for t in range(3):
                    nc.tensor.matmul(
                        accv[t][:, 0:OL] if t == 0 else accv[t][:],
                        xts[e][:, t * 128:(t + 1) * 128],
                        wts["v"][e][:],
                        start=(e == 0), stop=(e == NET - 1),
                    )
            for t in range(3):
                nc.vector.tensor_copy(
                    vbs[t][:], accv[t][:, 0:OL] if t == 0 else accv[t][:])

        def out_dma(eo, qb, ysb):
            q = nc.sync if eo % 2 == 0 else nc.gpsimd
            q.dma_start(
                yT[eo * 128:(eo + 1) * 128, qb * QB:(qb + 1) * QB], ysb[:])

        def outproj_group(qb, eo, pool=None, tag="pj"):
            y = (pool or pj_p).tile([128, QB], F32, tag=tag, name="y")
            for p2 in range(4):
                nc.tensor.matmul(
                    y[:],
                    wos[p2][:, eo * 128:(eo + 1) * 128],
                    atts[qb][p2][:],
                    start=(p2 == 0), stop=(p2 == 3),
                )
            ysb = ys_p.tile([128, QB], BF16, tag="ys", name="ys")
            nc.vector.tensor_copy(ysb[:], y[:])
            out_dma(eo, qb, ysb)

        # last q-block's outproj is split: the quad0 half (pair-tiles
        # 0,1 are done by ~step 130) runs as fillers during quad1-qb0;
        # the tail only streams the quad1 half and adds the saved half
        ya_sbs = [None] * NET

        def outproj3_a(eo):
            y = pj_p.tile([128, QB], F32, tag="pj", name="ya")
            for p2 in range(2):
                nc.tensor.matmul(
                    y[:],
                    wos[p2][:, eo * 128:(eo + 1) * 128],
                    atts[3][p2][:],
                    start=(p2 == 0), stop=(p2 == 1),
                )
            ya = ya_p.tile([128, QB], BF16, tag="ya", name="ya_sb")
            nc.vector.tensor_copy(ya[:], y[:])
            ya_sbs[eo] = ya

        def outproj3_b(eo):
            y = sc_p.tile([128, QB], F32, tag="sc", name="yb")
            for p2 in range(2, 4):
                nc.tensor.matmul(
                    y[:],
                    wos[p2][:, eo * 128:(eo + 1) * 128],
                    atts[3][p2][:],
                    start=(p2 == 2), stop=(p2 == 3),
                )
            ysb = ys_p.tile([128, QB], BF16, tag="ys", name="ys")
            nc.vector.tensor_add(ysb[:], ya_sbs[eo][:], y[:])
            out_dma(eo, 3, ysb)

        # ---- filler schedule ----
        def qg(i, j):
            return lambda: qk_group(i, j, "q")

        def kg(i, j):
            return lambda: qk_group(i, j, "k")

        def vg(t):
            return lambda: v_group(t)

        def og(qb, e):
            return lambda: outproj_group(qb, e)

        # Flat software-pipelined step list: one step per (quad, qb,
        # pair-in-quad, kt). At step i the ScalarE exp for step i is
        # emitted first, then the scores matmuls for step i+1, then PE
        # filler groups, then the U / sumexp matmuls for step i (which
        # wait on exp i) -- so ScalarE always has its next input queued.
        step_list = []
        for quad in range(2):
            for qb in range(NQB):
                for pi in range(2):
                    for kt in range(NKT):
                        step_list.append((quad, qb, 2 * quad + pi, pi, kt))
        nsteps = len(step_list)

        def sidx(quad, qb, pi, kt):
            return ((quad * NQB + qb) * 2 + pi) * NKT + kt

        sched = {}

        def put(step, th):
            sched.setdefault(step, []).append(th)

        for t in range(3, NKT):          # V(t) needed at step t
            put(t - 3, vg(t))
        for j in range(1, 4):            # K(0,j) read from step 4j-1
            put(4 * j - 4, kg(0, j))
        put(12, qg(1, 0))                # pair 1 starts at step 16
        put(13, kg(1, 0))
        for j in range(1, 4):            # K(1,j) read from step 16+4j-1
            put(16 + 4 * j - 4, kg(1, j))
        put(sidx(0, 0, 1, 12), qg(0, 1))   # Q(0,1) by step 32
        put(sidx(0, 1, 0, 12), qg(1, 1))   # Q(1,1) by step 48
        put(sidx(0, 1, 1, 4), qg(0, 2))    # Q(0,2) by step 64
        put(sidx(0, 1, 1, 12), qg(1, 2))
        put(sidx(0, 2, 0, 4), qg(0, 3))
        put(sidx(0, 2, 0, 12), qg(1, 3))
        # quad1 K/Q: needed from step 128 on. kg(2,*) sit exactly on
        # the quad0 group-end steps: those steps otherwise have no
        # filler, so the PE queue reaches the sumexp matmuls ~1.1us
        # before the DVE exp-accumulate chain finishes (16 stalls)
        put(sidx(0, 0, 1, 15), kg(2, 0))
        put(sidx(0, 1, 1, 15), kg(2, 1))
        put(sidx(0, 2, 1, 15), kg(2, 2))
        put(sidx(0, 3, 1, 15), kg(2, 3))
        put(sidx(0, 3, 0, 0), kg(3, 0))
        put(sidx(0, 3, 0, 4), kg(3, 1))
        put(sidx(0, 3, 0, 8), kg(3, 2))
        put(sidx(0, 3, 0, 12), kg(3, 3))
        put(sidx(0, 3, 1, 0), qg(2, 0))
        put(sidx(0, 3, 1, 8), qg(3, 0))
        put(sidx(1, 0, 0, 0), qg(2, 1))
        put(sidx(1, 0, 0, 8), qg(3, 1))
        put(sidx(1, 0, 1, 0), qg(2, 2))
        put(sidx(1, 0, 1, 8), qg(3, 2))
        put(sidx(1, 1, 0, 0), qg(2, 3))
        put(sidx(1, 1, 0, 8), qg(3, 3))
        for e in range(NET):             # outproj(qb) in (quad1, qb+1)
            # e=7 lands on the group-end step (kt15) as stall cover
            kt_e = 15 if e == 7 else 2 + 4 * (e // 2)
            put(sidx(1, 1, e % 2, kt_e), og(0, e))
            put(sidx(1, 2, e % 2, kt_e), og(1, e))
            put(sidx(1, 3, e % 2, 2 + 4 * (e // 2)), og(2, e))
        og3a_slots = [(0, 6), (0, 10), (0, 12), (0, 14),
                      (1, 2), (1, 6), (1, 10), (1, 15)]
        for e, (p, k) in enumerate(og3a_slots):
            put(sidx(1, 0, p, k),
                (lambda eo: (lambda: outproj3_a(eo)))(e))


        prologue_interleaved()

        # ---- attention ----
        def emit_scores(quad, qb, pair, kt):
            sc = sc_p.tile([128, 2 * QB], F32, tag="sc", name="sc")
            nc.tensor.matmul(
                sc[:, 0:QB],
                kts[pair][0:64, kt * 128:(kt + 1) * 128],
                qts[pair][0:64, qb * QB:(qb + 1) * QB],
                start=True, stop=True, tile_position=(0, 0),
            )
            nc.tensor.matmul(
                sc[:, QB:2 * QB],
                kts[pair][64:128, kt * 128:(kt + 1) * 128],
                qts[pair][64:128, qb * QB:(qb + 1) * QB],
                start=True, stop=True, tile_position=(64, 0),
            )
            return sc

        def norm_pair(quad, qb, pr, u2, acc):
            """Reduce one pair's exp-sum (2 M=1 matmuls from the DVE-
            accumulated tile) and normalize its U into atts.
            (gpsimd partition_all_reduce measured 6.7us/op on HW --
            4x the model -- so the PE ones-matmul stays.)"""
            seb = se_p.tile([64, QB], F32, tag="se", name="seb")
            for sub in range(2):
                nc.tensor.matmul(
                    seb[sub * 32:sub * 32 + 1, :],
                    onecol[:],
                    acc[:, sub * QB:(sub + 1) * QB],
                    start=True, stop=True,
                    tile_position=(0, sub * 32),
                    skip_group_check=True,
                )
            for sub in range(2):
                # stage Z to SBUF on ScalarE (Copy shares the exp
                # activation table, so no table reload); the custom-DVE
                # reciprocal misreads PSUM operands (~3% error) so it
                # must read SBUF
                rcs = nrm_p.tile([1, QB], F32, tag="rcs", name="rcs")
                nc.scalar.copy(rcs[:], seb[sub * 32:sub * 32 + 1, :])
                rcr = nrm_p.tile([1, QB], F32, tag="rcr", name="rcr")
                nc.vector.reciprocal_approx_fast(rcr[:], rcs[:])
                bcf = nrm_p.tile([128, QB], F32, tag="bcf", name="bcf")
                nc.gpsimd.partition_broadcast(bcf[:], rcr[:])
                nc.vector.tensor_mul(
                    atts[qb][pr][sub * 64:(sub + 1) * 64, :],
                    u2[sub * 64:(sub + 1) * 64, :],
                    bcf[sub * 64:(sub + 1) * 64, :])

        q0, q1, p1, _, k1 = step_list[0]
        pend_sc = emit_scores(q0, q1, p1, k1)
        cur = {}      # per-(quad,qb) state: uA, uB, acc0, acc1
        for i, (quad, qb, pair, pi, kt) in enumerate(step_list):
            if (pi, kt) == (0, 0):
                cur["uA"] = u_p.tile([128, QB], F32, tag="u", name="uA")
                cur["uB"] = u_p.tile([128, QB], F32, tag="u", name="uB")
            # exp for this step
            et = ex_p.tile([128, 2 * QB], BF16, tag="ex", name="ex")
            nc.scalar.activation(
                et[:], pend_sc[:],
                mybir.ActivationFunctionType.Exp, scale=0.125)
            # scores for next step
            if i + 1 < nsteps:
                nq, nqb, npair, _, nkt = step_list[i + 1]
                pend_sc = emit_scores(nq, nqb, npair, nkt)
            # fillers
            for th in sched.get(i, []):
                th()
            # U matmuls for this step
            u = cur["uA"] if pi == 0 else cur["uB"]
            for sub in range(2):
                hcol = (pair * 2 + sub) * D
                nc.tensor.matmul(
                    u[sub * 64:(sub + 1) * 64, :],
                    vbs[kt][:, hcol:hcol + D],
                    et[:, sub * QB:(sub + 1) * QB],
                    start=(kt == 0), stop=(kt == NKT - 1),
                    tile_position=(0, sub * 64),
                    skip_group_check=True,
                )
            # sumexp: accumulate exp tiles on DVE (frees the PE from
            # 16x M=1 ones-matmul streams per pair; one small matmul
            # per pair at group end reduces the accumulated tile)
            acck = "acc1" if pi else "acc0"
            if kt == 0:
                acc = ac_p.tile([128, 2 * QB], BF16, tag="ac",
                                name=acck)
                cur[acck] = acc
                nc.vector.tensor_copy(acc[:], et[:])
            else:
                acc = cur[acck]
                nc.vector.tensor_add(acc[:], acc[:], et[:])
            if pi == 1 and kt == 0:
                # pair A's U is complete: evacuate it so its PSUM bank
                # frees long before the next q-block needs it
                ua_sb = usb_p.tile([128, QB], BF16, tag="usb", name="ua_sb")
                nc.vector.tensor_copy(ua_sb[:], cur["uA"][:])
                cur["uA_sb"] = ua_sb
                # pair A's exp sum (acc0) is also complete: reduce +
                # normalize pair A now, spreading the group-end chain
                # across the pi=1 half instead of serializing at kt=15
                norm_pair(quad, qb, 2 * quad, ua_sb, cur["acc0"])
            if pi == 1 and kt == NKT - 1:
                norm_pair(quad, qb, 2 * quad + 1, cur["uB"],
                          cur["acc1"])
        # tail: only the quad1 half of the last q-block's outproj
        # remains (scores pool is free by now -- its banks pipeline)
        for eo in range(NET):
            outproj3_b(eo)

    nc.compile()
    return nc


_CACHED = {}


def _get_program():
    if "nc" not in _CACHED:
        _CACHED["nc"] = build_program()
    return _CACHED["nc"]


def make_inputs(embeddings, wq, bq, wk, bk, wv, bv, wo, bo):
    """Host-side sharding: per-core input maps."""
    in_maps = []
    for c in range(N_CORES):
        b, half = c // 2, c % 2
        sl = slice(half * OL, (half + 1) * OL)
        in_maps.append({
            "xT": np.ascontiguousarray(embeddings[b].T).astype(NPBF16),
            "wqT": np.ascontiguousarray(wq[sl, :].T).astype(NPBF16),
            "wkT": np.ascontiguousarray(wk[sl, :].T).astype(NPBF16),
            "wvT": np.ascontiguousarray(wv[sl, :].T).astype(NPBF16),
            "woT": np.ascontiguousarray(wo[:, sl].T).astype(NPBF16),
            "bqc": np.ascontiguousarray(
                bq[sl].reshape(4, 128).T).astype(np.float32),
            "bkc": np.ascontiguousarray(
                bk[sl].reshape(4, 128).T).astype(np.float32),
        })
    return in_maps


def unshard(results, bo_eff):
    out = np.empty((B, S, E), np.float32)
    for b in range(B):
        yt = (results[2 * b]["yT"].astype(np.float32)
              + results[2 * b + 1]["yT"].astype(np.float32))
        out[b] = yt.T + bo_eff[None, :]
    return out


def kernel(embeddings, wq, bq, wk, bk, wv, bv, wo, bo, _trace=False):
    embeddings = np.asarray(embeddings, np.float32)
    wv = np.asarray(wv, np.float32)
    bv = np.asarray(bv, np.float32)
    wo = np.asarray(wo, np.float32)
    bo = np.asarray(bo, np.float32)
    # V bias passes through softmax (rows sum to 1): fold into bo.
    bo_eff = bo + wo @ bv
    nc = _get_program()
    in_maps = make_inputs(
        embeddings, np.asarray(wq, np.float32), np.asarray(bq, np.float32),
        np.asarray(wk, np.float32), np.asarray(bk, np.float32),
        wv, bv, wo, bo)
    res = run_bass_kernel_spmd(
        nc, in_maps, core_ids=list(range(N_CORES)), trace=_trace)
    out = unshard(res.results, bo_eff)
    if _trace:
        kernel.last_result = res
    return out



# revision 53
# speedup vs baseline: 1.0094x; 1.0021x over previous
"""Multi-head attention kernel for 8 Trainium2 NeuronCores.

Problem: embeddings [4, 2048, 1024], 16 heads x 64 dim, torch nn.Linear
convention (x @ W.T + b) for Q/K/V/O projections.

Sharding: batch (4) x head-halves (2) -> 8 cores. Core c handles batch
c//2, local heads (c%2)*8..(c%2)*8+8. Output projection is row-sharded;
host sums the two partial outputs per batch element and adds the
effective output bias bo' = bo + wo @ bv: softmax rows sum to 1, so
the V bias passes through attention unchanged and is folded into the
output bias on the host (16 per-core bias matmuls + one DMA deleted).

Per-core dataflow (feature dims on partitions; PE stream columns are
the wall at ~2.35 cols/ns, so every column saved counts):
  xT [1024e, 2048t] bf16 (host pre-transposed + cast)
  QT/KT [(h,d)=512, t] via PE, bias added on DVE during PSUM evac.
  V [t, (h,d)] via PE (no bias; folded into bo on the host).
  Input DMAs split per first-use chunk across the sync/scalar/gpsimd
  issue queues (issue rate ~0.6-1us/DMA is the startup constraint; a
  DMA's packets spread over all 16 engines). The first projection
  groups run with their PSUM chains interleaved across idle banks.
  Per head-quad (4 heads = 2 pairs), per q-block of 512, per k-tile:
    scores_T[k,q] row-paired matmuls (2 heads share the PE array),
    exp on ScalarE (1/8 scale folded in, no max subtraction needed),
    U[(2x64),q] col-paired matmuls; exp tiles accumulate on DVE
    (bf16 in-place adds) so sumexp costs PE only 2 M=1 matmuls per
    pair at group end instead of 16x512-col ones-matmul streams.
  normalize: ScalarE-copy of each sumexp row to SBUF (the custom-DVE
  reciprocal misreads PSUM operands at ~3% error; gpsimd
  partition_all_reduce measured 6.7us/op -- too slow) -> recip ->
  gpsimd partition-broadcast -> DVE mult (deferring pair B's
  normalize into the next group was tried and regressed ~6us: the
  stall just moves into the outproj fillers).
  yT[e_out, t] = woT.T @ attn_T accumulated over 4 pair-tiles; the
  last q-block's outproj is split: its quad0 half runs as
  quad1-qb0 fillers, so the tail streams only the quad1 half plus
  a DVE add. yT is bf16 (host sums the two row-shard halves in
  fp32).
Host: out[b] = (yT[2b] + yT[2b+1]).T + bo'.
"""

import sys

sys.path.insert(0, "/opt/trn_rl_repo")

import numpy as np
import ml_dtypes

import concourse.bass as bass
import concourse.bacc as bacc
import concourse.mybir as mybir
import concourse.bass_isa as bass_isa
import concourse.tile as tile
from concourse.bass_utils import run_bass_kernel_spmd

BF16 = mybir.dt.bfloat16
F32 = mybir.dt.float32
NPBF16 = ml_dtypes.bfloat16

B, S, E = 4, 2048, 1024
H_LOC = 8          # local heads per core
D = 64             # head dim
OL = H_LOC * D     # 512 local output dim
N_CORES = 8
QB = 512           # query block (free dim of scores_T)
NQB = S // QB      # 4
NKT = S // 128     # 16 key tiles
NET = E // 128     # 8 embed tiles


def build_program():
    from contextlib import ExitStack

    nc = bacc.Bacc("TRN2", debug=False, num_devices=N_CORES)

    xT = nc.dram_tensor("xT", [E, S], BF16, kind="ExternalInput")
    wqT = nc.dram_tensor("wqT", [E, OL], BF16, kind="ExternalInput")
    wkT = nc.dram_tensor("wkT", [E, OL], BF16, kind="ExternalInput")
    wvT = nc.dram_tensor("wvT", [E, OL], BF16, kind="ExternalInput")
    woT = nc.dram_tensor("woT", [OL, E], BF16, kind="ExternalInput")
    bqc = nc.dram_tensor("bqc", [128, 4], F32, kind="ExternalInput")
    bkc = nc.dram_tensor("bkc", [128, 4], F32, kind="ExternalInput")
    yT = nc.dram_tensor("yT", [E, S], BF16, kind="ExternalOutput")

    with tile.TileContext(nc) as tc, ExitStack() as est:
        xt_p = est.enter_context(tc.tile_pool(name="xt", bufs=NET))
        wq_p = est.enter_context(tc.tile_pool(name="wq", bufs=NET))
        wk_p = est.enter_context(tc.tile_pool(name="wk", bufs=NET))
        wv_p = est.enter_context(tc.tile_pool(name="wv", bufs=NET))
        wo_p = est.enter_context(tc.tile_pool(name="wo", bufs=4))
        bias_p = est.enter_context(tc.tile_pool(name="bias", bufs=4))
        qt_p = est.enter_context(tc.tile_pool(name="qt", bufs=4))
        kt_p = est.enter_context(tc.tile_pool(name="kt", bufs=4))
        vb_p = est.enter_context(tc.tile_pool(name="vb", bufs=NKT))
        pj_p = est.enter_context(tc.tile_pool(name="pj", bufs=1, space="PSUM"))
        sc_p = est.enter_context(tc.tile_pool(name="sc", bufs=2, space="PSUM"))
        u_p = est.enter_context(tc.tile_pool(name="u", bufs=2, space="PSUM"))
        se_p = est.enter_context(tc.tile_pool(name="se", bufs=1, space="PSUM"))
        ex_p = est.enter_context(tc.tile_pool(name="ex", bufs=20))
        ac_p = est.enter_context(tc.tile_pool(name="ac", bufs=4))
        at_p = est.enter_context(tc.tile_pool(name="at", bufs=16))
        nrm_p = est.enter_context(tc.tile_pool(name="nrm", bufs=2))
        ys_p = est.enter_context(tc.tile_pool(name="ys", bufs=2))
        ya_p = est.enter_context(tc.tile_pool(name="ya", bufs=NET))
        usb_p = est.enter_context(tc.tile_pool(name="usb", bufs=4))

        # ---- load inputs ----
        # Issue-order + queue split so the first projection groups can
        # start ~8us in (one queue serializes issue at ~0.6-1us/DMA;
        # a DMA's packets spread across all 16 engines, so many small
        # DMAs in flight saturate HBM):
        #   sync:   biases, x j0-chunks, x rest, wq/wk col-rest
        #   scalar: wq/wk first col-chunk (before any exp work)
        #   gpsimd: wv (full), wo; output tiles later
        xts = [xt_p.tile([128, S], BF16, tag="xt", name="xt")
               for _ in range(NET)]
        wts = {
            name: [pool.tile([128, OL], BF16, tag="w" + name,
                             name="w" + name) for _ in range(NET)]
            for name, pool in (("q", wq_p), ("k", wk_p), ("v", wv_p))
        }
        wos = [wo_p.tile([128, E], BF16, tag="wo", name="wo")
               for _ in range(4)]
        bqs = bias_p.tile([128, 4], F32, tag="bqc")
        bks = bias_p.tile([128, 4], F32, tag="bkc")
        onecol = bias_p.tile([128, 1], BF16, tag="onecol")
        nc.vector.memset(onecol[:], 1.0)
        nc.sync.dma_start(bqs[:], bqc[:])
        nc.sync.dma_start(bks[:], bkc[:])
        # PE clock warmup: the engine idles ~11us waiting for input
        # DMAs and then runs the whole projection phase at the low
        # p-state (~600ns/512-col matmul vs 215ns hot). Burn dummy
        # matmuls on memset scratch during the DMA window so the real
        # prologue starts on a hot clock.
        wsrc = bias_p.tile([128, QB], BF16, tag="wsrc")
        nc.vector.memset(wsrc[:], 0.0)
        wdst = se_p.tile([128, QB], F32, tag="se", name="warmmm")
        for _ in range(24):
            nc.tensor.matmul(wdst[0:1, :], onecol[:], wsrc[:],
                             start=True, stop=True)
        # Preload the exp activation table while input DMAs run.
        warm = bias_p.tile([1, 16], F32, tag="warm")
        nc.vector.memset(warm[:], 0.0)
        warm2 = bias_p.tile([1, 16], F32, tag="warm2")
        nc.scalar.activation(warm2[:], warm[:],
                             mybir.ActivationFunctionType.Exp)
        # queue-priority order matches first use: prologue needs
        # x_j0+wq_i0+wk_i0 (~13us), wv by ~18us, x_j1..3 by steps 0-12,
        # wq/wk_i1 by step ~13, i2/i3 by ~step 36+, wo much later
        for e in range(NET):
            r = slice(e * 128, (e + 1) * 128)
            nc.scalar.dma_start(wts["q"][e][:, 0:128], wqT[r, 0:128])
        for e in range(NET):
            r = slice(e * 128, (e + 1) * 128)
            nc.scalar.dma_start(wts["k"][e][:, 0:128], wkT[r, 0:128])
        for e in range(NET):
            r = slice(e * 128, (e + 1) * 128)
            nc.gpsimd.dma_start(wts["v"][e][:], wvT[r, :])
        for j in range(4):
            for e in range(NET):
                r = slice(e * 128, (e + 1) * 128)
                nc.sync.dma_start(xts[e][:, j * QB:(j + 1) * QB],
                                  xT[r, j * QB:(j + 1) * QB])
        for e in range(NET):
            r = slice(e * 128, (e + 1) * 128)
            nc.gpsimd.dma_start(wts["q"][e][:, 128:256], wqT[r, 128:256])
        for e in range(NET):
            r = slice(e * 128, (e + 1) * 128)
            nc.gpsimd.dma_start(wts["k"][e][:, 128:256], wkT[r, 128:256])
        for e in range(NET):
            r = slice(e * 128, (e + 1) * 128)
            nc.sync.dma_start(wts["q"][e][:, 256:OL], wqT[r, 256:OL])
            nc.sync.dma_start(wts["k"][e][:, 256:OL], wkT[r, 256:OL])
        for p in range(4):
            nc.sync.dma_start(wos[p][:], woT[p * 128:(p + 1) * 128, :])

        qts = [qt_p.tile([128, S], BF16, tag="qt", name="qt")
               for _ in range(4)]
        kts = [kt_p.tile([128, S], BF16, tag="kt", name="kt")
               for _ in range(4)]
        vbs = [vb_p.tile([128, OL], BF16, tag="vb", name="vb")
               for _ in range(NKT)]
        atts = [[at_p.tile([128, QB], BF16, tag="at", name="at")
                 for _ in range(4)] for _ in range(NQB)]

        # ---- projection / outproj group emitters (PE fillers) ----
        def qk_group(i, j, which):
            """Q or K projection for o-tile i, t-block j (one PSUM group)."""
            w = wts[which]
            bias_t = bqs if which == "q" else bks
            dest = qts[i] if which == "q" else kts[i]
            acc = pj_p.tile([128, QB], F32, tag="pj", name="pj")
            for e in range(NET):
                nc.tensor.matmul(
                    acc[:],
                    w[e][:, i * 128:(i + 1) * 128],
                    xts[e][:, j * QB:(j + 1) * QB],
                    start=(e == 0), stop=(e == NET - 1),
                )
            nc.vector.tensor_scalar_add(
                dest[:, j * QB:(j + 1) * QB], acc[:], bias_t[:, i:i + 1])

        def v_group(ti):
            acc = pj_p.tile([128, OL], F32, tag="pj", name="pjv")
            for e in range(NET):
                nc.tensor.matmul(
                    acc[:],
                    xts[e][:, ti * 128:(ti + 1) * 128],
                    wts["v"][e][:],
                    start=(e == 0), stop=(e == NET - 1),
                )
            nc.vector.tensor_copy(vbs[ti][:], acc[:])

        def prologue_interleaved():
            """First groups with their PSUM chains interleaved across
            banks (sc/u pools are idle before attention starts), so
            matmuls from independent chains pipeline while the input
            DMAs are still streaming."""
            accq = pj_p.tile([128, QB], F32, tag="pj", name="pj")
            acck = sc_p.tile([128, 2 * QB], F32, tag="sc", name="sc")
            for e in range(NET):
                nc.tensor.matmul(
                    accq[:], wts["q"][e][:, 0:128], xts[e][:, 0:QB],
                    start=(e == 0), stop=(e == NET - 1))
                nc.tensor.matmul(
                    acck[:, 0:QB], wts["k"][e][:, 0:128],
                    xts[e][:, 0:QB],
                    start=(e == 0), stop=(e == NET - 1))
            nc.vector.tensor_scalar_add(
                qts[0][:, 0:QB], accq[:], bqs[:, 0:1])
            nc.vector.tensor_scalar_add(
                kts[0][:, 0:QB], acck[:, 0:QB], bks[:, 0:1])
            accv = [sc_p.tile([128, 2 * QB], F32, tag="sc", name="sc"),
                    u_p.tile([128, QB], F32, tag="u", name="uA"),
                    u_p.tile([128, QB], F32, tag="u", name="uB")]
            for e in range(NET):
                for t in range(3):
                    nc.tensor.matmul(
                        accv[t][:, 0:OL] if t == 0 else accv[t][:],
                        xts[e][:, t * 128:(t + 1) * 128],
                        wts["v"][e][:],
                        start=(e == 0), stop=(e == NET - 1),
                    )
            for t in range(3):
                nc.vector.tensor_copy(
                    vbs[t][:], accv[t][:, 0:OL] if t == 0 else accv[t][:])

        def out_dma(eo, qb, ysb):
            # the last q-block's tiles go on sync only: a trailing
            # SWDGE DMA adds ~3.7us to the gpsimd drain at NEFF end
            q = nc.sync if (qb == 3 or eo % 2 == 0) else nc.gpsimd
            q.dma_start(
                yT[eo * 128:(eo + 1) * 128, qb * QB:(qb + 1) * QB], ysb[:])

        def outproj_group(qb, eo, pool=None, tag="pj"):
            y = (pool or pj_p).tile([128, QB], F32, tag=tag, name="y")
            for p2 in range(4):
                nc.tensor.matmul(
                    y[:],
                    wos[p2][:, eo * 128:(eo + 1) * 128],
                    atts[qb][p2][:],
                    start=(p2 == 0), stop=(p2 == 3),
                )
            ysb = ys_p.tile([128, QB], BF16, tag="ys", name="ys")
            nc.vector.tensor_copy(ysb[:], y[:])
            out_dma(eo, qb, ysb)

        # last q-block's outproj is split: the quad0 half (pair-tiles
        # 0,1 are done by ~step 130) runs as fillers during quad1-qb0;
        # the tail only streams the quad1 half and adds the saved half
        ya_sbs = [None] * NET

        def outproj3_a(eo):
            y = pj_p.tile([128, QB], F32, tag="pj", name="ya")
            for p2 in range(2):
                nc.tensor.matmul(
                    y[:],
                    wos[p2][:, eo * 128:(eo + 1) * 128],
                    atts[3][p2][:],
                    start=(p2 == 0), stop=(p2 == 1),
                )
            ya = ya_p.tile([128, QB], BF16, tag="ya", name="ya_sb")
            nc.vector.tensor_copy(ya[:], y[:])
            ya_sbs[eo] = ya

        def outproj3_b(eo):
            y = sc_p.tile([128, QB], F32, tag="sc", name="yb")
            for p2 in range(2, 4):
                nc.tensor.matmul(
                    y[:],
                    wos[p2][:, eo * 128:(eo + 1) * 128],
                    atts[3][p2][:],
                    start=(p2 == 2), stop=(p2 == 3),
                )
            ysb = ys_p.tile([128, QB], BF16, tag="ys", name="ys")
            nc.vector.tensor_add(ysb[:], ya_sbs[eo][:], y[:])
            out_dma(eo, 3, ysb)

        # ---- filler schedule ----
        def qg(i, j):
            return lambda: qk_group(i, j, "q")

        def kg(i, j):
            return lambda: qk_group(i, j, "k")

        def vg(t):
            return lambda: v_group(t)

        def og(qb, e):
            return lambda: outproj_group(qb, e)

        # Flat software-pipelined step list: one step per (quad, qb,
        # pair-in-quad, kt). At step i the ScalarE exp for step i is
        # emitted first, then the scores matmuls for step i+1, then PE
        # filler groups, then the U / sumexp matmuls for step i (which
        # wait on exp i) -- so ScalarE always has its next input queued.
        step_list = []
        for quad in range(2):
            for qb in range(NQB):
                for pi in range(2):
                    for kt in range(NKT):
                        step_list.append((quad, qb, 2 * quad + pi, pi, kt))
        nsteps = len(step_list)

        def sidx(quad, qb, pi, kt):
            return ((quad * NQB + qb) * 2 + pi) * NKT + kt

        sched = {}

        def put(step, th):
            sched.setdefault(step, []).append(th)

        for t in range(3, NKT):          # V(t) needed at step t
            put(t - 3, vg(t))
        for j in range(1, 4):            # K(0,j) read from step 4j-1
            put(4 * j - 4, kg(0, j))
        put(12, qg(1, 0))                # pair 1 starts at step 16
        put(13, kg(1, 0))
        for j in range(1, 4):            # K(1,j) read from step 16+4j-1
            put(16 + 4 * j - 4, kg(1, j))
        put(sidx(0, 0, 1, 12), qg(0, 1))   # Q(0,1) by step 32
        put(sidx(0, 1, 0, 12), qg(1, 1))   # Q(1,1) by step 48
        put(sidx(0, 1, 1, 4), qg(0, 2))    # Q(0,2) by step 64
        put(sidx(0, 1, 1, 12), qg(1, 2))
        put(sidx(0, 2, 0, 4), qg(0, 3))
        put(sidx(0, 2, 0, 12), qg(1, 3))
        # quad1 K/Q: needed from step 128 on. kg(2,*) sit exactly on
        # the quad0 group-end steps: those steps otherwise have no
        # filler, so the PE queue reaches the sumexp matmuls ~1.1us
        # before the DVE exp-accumulate chain finishes (16 stalls)
        put(sidx(0, 0, 1, 15), kg(2, 0))
        put(sidx(0, 1, 1, 15), kg(2, 1))
        put(sidx(0, 2, 1, 15), kg(2, 2))
        put(sidx(0, 3, 1, 15), kg(2, 3))
        put(sidx(0, 3, 0, 0), kg(3, 0))
        put(sidx(0, 3, 0, 4), kg(3, 1))
        put(sidx(0, 3, 0, 8), kg(3, 2))
        put(sidx(0, 3, 0, 12), kg(3, 3))
        put(sidx(0, 3, 1, 0), qg(2, 0))
        put(sidx(0, 3, 1, 8), qg(3, 0))
        put(sidx(1, 0, 0, 0), qg(2, 1))
        put(sidx(1, 0, 0, 8), qg(3, 1))
        put(sidx(1, 0, 1, 0), qg(2, 2))
        put(sidx(1, 0, 1, 8), qg(3, 2))
        put(sidx(1, 1, 0, 0), qg(2, 3))
        put(sidx(1, 1, 0, 8), qg(3, 3))
        for e in range(NET):             # outproj(qb) in (quad1, qb+1)
            # e=7 lands on the group-end step (kt15) as stall cover
            kt_e = 15 if e == 7 else 2 + 4 * (e // 2)
            put(sidx(1, 1, e % 2, kt_e), og(0, e))
            put(sidx(1, 2, e % 2, kt_e), og(1, e))
            put(sidx(1, 3, e % 2, 2 + 4 * (e // 2)), og(2, e))
        og3a_slots = [(0, 6), (0, 10), (0, 12), (0, 14),
                      (1, 2), (1, 6), (1, 10), (1, 15)]
        for e, (p, k) in enumerate(og3a_slots):
            put(sidx(1, 0, p, k),
                (lambda eo: (lambda: outproj3_a(eo)))(e))


        prologue_interleaved()

        # ---- attention ----
        def emit_scores(quad, qb, pair, kt):
            sc = sc_p.tile([128, 2 * QB], F32, tag="sc", name="sc")
            nc.tensor.matmul(
                sc[:, 0:QB],
                kts[pair][0:64, kt * 128:(kt + 1) * 128],
                qts[pair][0:64, qb * QB:(qb + 1) * QB],
                start=True, stop=True, tile_position=(0, 0),
            )
            nc.tensor.matmul(
                sc[:, QB:2 * QB],
                kts[pair][64:128, kt * 128:(kt + 1) * 128],
                qts[pair][64:128, qb * QB:(qb + 1) * QB],
                start=True, stop=True, tile_position=(64, 0),
            )
            return sc

        def norm_pair(quad, qb, pr, u2, acc):
            """Reduce one pair's exp-sum (2 M=1 matmuls from the DVE-
            accumulated tile) and normalize its U into atts.
            (gpsimd partition_all_reduce measured 6.7us/op on HW --
            4x the model -- so the PE ones-matmul stays.)"""
            seb = se_p.tile([64, QB], F32, tag="se", name="seb")
            for sub in range(2):
                nc.tensor.matmul(
                    seb[sub * 32:sub * 32 + 1, :],
                    onecol[:],
                    acc[:, sub * QB:(sub + 1) * QB],
                    start=True, stop=True,
                    tile_position=(0, sub * 32),
                    skip_group_check=True,
                )
            for sub in range(2):
                # stage Z to SBUF on ScalarE (Copy shares the exp
                # activation table, so no table reload); the custom-DVE
                # reciprocal misreads PSUM operands (~3% error) so it
                # must read SBUF
                rcs = nrm_p.tile([1, QB], F32, tag="rcs", name="rcs")
                nc.scalar.copy(rcs[:], seb[sub * 32:sub * 32 + 1, :])
                rcr = nrm_p.tile([1, QB], F32, tag="rcr", name="rcr")
                nc.vector.reciprocal_approx_fast(rcr[:], rcs[:])
                bcf = nrm_p.tile([128, QB], F32, tag="bcf", name="bcf")
                nc.gpsimd.partition_broadcast(bcf[:], rcr[:])
                nc.vector.tensor_mul(
                    atts[qb][pr][sub * 64:(sub + 1) * 64, :],
                    u2[sub * 64:(sub + 1) * 64, :],
                    bcf[sub * 64:(sub + 1) * 64, :])

        q0, q1, p1, _, k1 = step_list[0]
        pend_sc = emit_scores(q0, q1, p1, k1)
        cur = {}      # per-(quad,qb) state: uA, uB, acc0, acc1
        for i, (quad, qb, pair, pi, kt) in enumerate(step_list):
            if (pi, kt) == (0, 0):
                cur["uA"] = u_p.tile([128, QB], F32, tag="u", name="uA")
                cur["uB"] = u_p.tile([128, QB], F32, tag="u", name="uB")
            # exp for this step
            et = ex_p.tile([128, 2 * QB], BF16, tag="ex", name="ex")
            nc.scalar.activation(
                et[:], pend_sc[:],
                mybir.ActivationFunctionType.Exp, scale=0.125)
            # scores for next step
            if i + 1 < nsteps:
                nq, nqb, npair, _, nkt = step_list[i + 1]
                pend_sc = emit_scores(nq, nqb, npair, nkt)
            # fillers
            for th in sched.get(i, []):
                th()
            # U matmuls for this step
            u = cur["uA"] if pi == 0 else cur["uB"]
            for sub in range(2):
                hcol = (pair * 2 + sub) * D
                nc.tensor.matmul(
                    u[sub * 64:(sub + 1) * 64, :],
                    vbs[kt][:, hcol:hcol + D],
                    et[:, sub * QB:(sub + 1) * QB],
                    start=(kt == 0), stop=(kt == NKT - 1),
                    tile_position=(0, sub * 64),
                    skip_group_check=True,
                )
            # sumexp: accumulate exp tiles on DVE (frees the PE from
            # 16x M=1 ones-matmul streams per pair; one small matmul
            # per pair at group end reduces the accumulated tile)
            acck = "acc1" if pi else "acc0"
            if kt == 0:
                acc = ac_p.tile([128, 2 * QB], BF16, tag="ac",
                                name=acck)
                cur[acck] = acc
                nc.vector.tensor_copy(acc[:], et[:])
            else:
                acc = cur[acck]
                nc.vector.tensor_add(acc[:], acc[:], et[:])
            if pi == 1 and kt == 0:
                # pair A's U is complete: evacuate it so its PSUM bank
                # frees long before the next q-block needs it
                ua_sb = usb_p.tile([128, QB], BF16, tag="usb", name="ua_sb")
                nc.vector.tensor_copy(ua_sb[:], cur["uA"][:])
                cur["uA_sb"] = ua_sb
                # pair A's exp sum (acc0) is also complete: reduce +
                # normalize pair A now, spreading the group-end chain
                # across the pi=1 half instead of serializing at kt=15
                norm_pair(quad, qb, 2 * quad, ua_sb, cur["acc0"])
            if pi == 1 and kt == NKT - 1:
                norm_pair(quad, qb, 2 * quad + 1, cur["uB"],
                          cur["acc1"])
        # tail: only the quad1 half of the last q-block's outproj
        # remains (scores pool is free by now -- its banks pipeline)
        for eo in range(NET):
            outproj3_b(eo)

    nc.compile()
    return nc


_CACHED = {}


def _get_program():
    if "nc" not in _CACHED:
        _CACHED["nc"] = build_program()
    return _CACHED["nc"]


def make_inputs(embeddings, wq, bq, wk, bk, wv, bv, wo, bo):
    """Host-side sharding: per-core input maps."""
    in_maps = []
    for c in range(N_CORES):
        b, half = c // 2, c % 2
        sl = slice(half * OL, (half + 1) * OL)
        in_maps.append({
            "xT": np.ascontiguousarray(embeddings[b].T).astype(NPBF16),
            "wqT": np.ascontiguousarray(wq[sl, :].T).astype(NPBF16),
            "wkT": np.ascontiguousarray(wk[sl, :].T).astype(NPBF16),
            "wvT": np.ascontiguousarray(wv[sl, :].T).astype(NPBF16),
            "woT": np.ascontiguousarray(wo[:, sl].T).astype(NPBF16),
            "bqc": np.ascontiguousarray(
                bq[sl].reshape(4, 128).T).astype(np.float32),
            "bkc": np.ascontiguousarray(
                bk[sl].reshape(4, 128).T).astype(np.float32),
        })
    return in_maps


def unshard(results, bo_eff):
    out = np.empty((B, S, E), np.float32)
    for b in range(B):
        yt = (results[2 * b]["yT"].astype(np.float32)
              + results[2 * b + 1]["yT"].astype(np.float32))
        out[b] = yt.T + bo_eff[None, :]
    return out


def kernel(embeddings, wq, bq, wk, bk, wv, bv, wo, bo, _trace=False):
    embeddings = np.asarray(embeddings, np.float32)
    wv = np.asarray(wv, np.float32)
    bv = np.asarray(bv, np.float32)
    wo = np.asarray(wo, np.float32)
    bo = np.asarray(bo, np.float32)
    # V bias passes through softmax (rows sum to 1): fold into bo.
    bo_eff = bo + wo @ bv
    nc = _get_program()
    in_maps = make_inputs(
        embeddings, np.asarray(wq, np.float32), np.asarray(bq, np.float32),
        np.asarray(wk, np.float32), np.asarray(bk, np.float32),
        wv, bv, wo, bo)
    res = run_bass_kernel_spmd(
        nc, in_maps, core_ids=list(range(N_CORES)), trace=_trace)
    out = unshard(res.results, bo_eff)
    if _trace:
        kernel.last_result = res
    return out

